# revision 9
# baseline (speedup 1.0000x reference)
"""Trainium2 Bass kernel for nn_ChannelMixing (RWKV-style channel mixing).

Math: the reference's FFT decay-conv is the first-order IIR
    h[t] = mix*h[t-1] + x[t],  h[-1] = last_x/(1-mix)
and x_mix = (1-mix)*h, so with weights pre-scaled by (1-mix):
    k = h_k @ (Wk*(1-mix_k)).T,  r = h_r @ (Wr*(1-mix_r)).T
    out = sigmoid(r) * (relu(k)^2 @ Wv.T)

Sharding: time L=4096 split over 8 cores (512 each) with a 32-step halo
to warm the scan state (decay <= sigmoid(1) ~ 0.731; carry error < 5e-5,
far below the bf16 noise floor). Core 0 gets the exact initial state via
a per-core init column; no collectives.

Precision: scans keep fp32 state on the DVE and write bf16 (k path) or
fp8-e4m3 (r path) outputs directly. Wk/Wv matmuls run bf16; Wr runs fp8
DoubleRow (2x PE throughput, contraction 256/instr); its fp8 error is
damped ~4x by the sigmoid gate. End-to-end rel err ~8e-3 vs 2e-2 gate.

Schedule: Wk runs as two kt-major half-passes over all 8 PSUM banks so
the PE paces with the scan cadence instead of starving; Wr/Wv are
group-major. x slabs ride the sync queue first, then wr8/wv weights
(batched 256-512KB, early-resident); wk weights + outputs ride scalar.
PE cadence is 259ns/512-col matmul on trn2 (2.0 GHz) — the matmul
stream itself is at the hardware floor; the schedule hides the rest.
"""
import numpy as np
import ml_dtypes
from contextlib import ExitStack

import concourse.bass as bass
from concourse import bacc
import concourse.tile as tile
import concourse.mybir as mybir
from concourse.bass_utils import run_bass_kernel_spmd

LEN, DIM = 4096, 2048
NCORES = 8
P = 128
HALO = 32
NT = DIM // P          # 16 channel tiles
TLOC = LEN // NCORES   # 512
TS = TLOC + HALO       # 544

f32 = mybir.dt.float32
bf16 = mybir.dt.bfloat16
fp8 = mybir.dt.float8e4
Alu = mybir.AluOpType
Act = mybir.ActivationFunctionType
DR = mybir.MatmulPerfMode.DoubleRow

_cache = {}


def _build():
    nc = bacc.Bacc(trn_type="TRN2", debug=False)

    # x pre-packed host-side into a [p, ct*TS] SBUF image: 17KB contiguous
    # rows -> large DMA packets (the [DIM, TS] layout shattered into 136B
    # packets and starved the scans).
    xs_d = nc.dram_tensor("xs", [P, NT * TS], bf16, kind="ExternalInput").ap()
    # cst image: cols [0:2*NT+2] = dec (2*ct = mix_k, 2*ct+1 = mix_r,
    # 2*NT = 1/s_w); cols [2*NT+2:] = scan init columns.
    cst_d = nc.dram_tensor("cst", [P, 4 * NT + 2], f32, kind="ExternalInput").ap()
    wk_d = nc.dram_tensor("wk", [DIM, DIM], bf16, kind="ExternalInput").ap()
    # wv pre-permuted host-side to [p, kt, o] so a [128, 4, 512] tile is one DMA.
    wv_d = nc.dram_tensor("wv", [P, NT, DIM], bf16, kind="ExternalInput").ap()
    # wr8 packed [kt4, p, j(4 c-subtiles), o] for fp8 DoubleRow.
    wr_d = nc.dram_tensor("wr8", [NT // 4, P, 4, DIM], fp8, kind="ExternalInput").ap()
    # out as [p, m, t]; host reassembles.
    out_d = nc.dram_tensor("out", [P, NT, TLOC], bf16, kind="ExternalOutput").ap()

    with tile.TileContext(nc) as tc, ExitStack() as ctx:
        const = ctx.enter_context(tc.tile_pool(name="const", bufs=1))
        xs_pool = ctx.enter_context(tc.tile_pool(name="xs", bufs=1))
        hk_pool = ctx.enter_context(tc.tile_pool(name="hk", bufs=1))
        hr_pool = ctx.enter_context(tc.tile_pool(name="hr", bufs=1))
        wk_pool = ctx.enter_context(tc.tile_pool(name="wkp", bufs=10))
        wv_pool = ctx.enter_context(tc.tile_pool(name="wvp", bufs=4))
        w8_pool = ctx.enter_context(tc.tile_pool(name="w8p", bufs=1))
        sq_pool = ctx.enter_context(tc.tile_pool(name="sq", bufs=1))
        sig_pool = ctx.enter_context(tc.tile_pool(name="sg", bufs=1))
        rr_pool = ctx.enter_context(tc.tile_pool(name="rr", bufs=1))
        o_pool = ctx.enter_context(tc.tile_pool(name="o", bufs=2))
        ps_pool = ctx.enter_context(tc.tile_pool(name="ps", bufs=1, space="PSUM"))

        # ---- PE warmup from a memset tile: no DMA dependency.
        wm = const.tile([P, 512], bf16, name="warm")
        nc.vector.memset(wm[:], 0.25)
        ps_w = ps_pool.tile([P, 512], f32, tag="p7", name="ps_warm")
        for _ in range(12):
            nc.tensor.matmul(ps_w[:], wm[:, 0:P], wm[:], start=True, stop=True)

        # dec+ini as one small DMA, first on sync: lands before the x slabs.
        cst_t = const.tile([P, 4 * NT + 2], f32)
        nc.sync.dma_start(cst_t[:], cst_d)
        dec_t = cst_t[:, 0:2 * NT + 2]
        ini_t = cst_t[:, 2 * NT + 2:]

        # ---- x slabs on sync, uneven split: tiny first chunk so scan 0
        # starts earliest; later chunks grow since scans consume 1/1.24us.
        XSPLIT = [1, 1, 2, 2, 2, 2, 3, 3]
        xs = []
        ct0 = 0
        for i, nslab in enumerate(XSPLIT):
            t = xs_pool.tile([P, nslab * TS], bf16, tag=f"xs{i}", name=f"xs{i}")
            nc.sync.dma_start(t[:], xs_d[:, ct0 * TS:(ct0 + nslab) * TS])
            for s in range(nslab):
                xs.append(t[:, s * TS:(s + 1) * TS])
            ct0 += nslab

        # wr8 weights: 16 tiles, early-resident (4MB), on sync after xs.
        w8 = []
        for g in range(4):
            for kt4 in range(NT // 4):
                t = w8_pool.tile([P, 4, 512], fp8, tag=f"w8_{g}_{kt4}",
                                 name=f"wr{g}_{kt4}")
                nc.sync.dma_start(t[:], wr_d[kt4, :, :, g * 512:(g + 1) * 512])
                w8.append(t)

        # ---- scans: fp32 state on DVE; k -> bf16, r -> fp8 DR-packed ----
        hk = []
        for ct in range(NT):
            t = hk_pool.tile([P, TS], bf16, tag=f"hk{ct}", name=f"hk{ct}")
            nc.vector.tensor_tensor_scan(
                t[:], dec_t[:, 2 * ct:2 * ct + 1].broadcast_to([P, TS]),
                xs[ct], ini_t[:, 2 * ct:2 * ct + 1],
                op0=Alu.mult, op1=Alu.add)
            hk.append(t)
        hr8 = [hr_pool.tile([P, 2, TS], fp8, tag=f"hr{i}", name=f"hr{i}")
               for i in range(NT // 2)]
        for ct in range(NT):
            c = 2 * ct + 1
            nc.vector.tensor_tensor_scan(
                hr8[ct // 2][:, ct % 2, :],
                dec_t[:, c:c + 1].broadcast_to([P, TS]),
                xs[ct], ini_t[:, c:c + 1],
                op0=Alu.mult, op1=Alu.add)

        sq = [sq_pool.tile([P, TLOC], bf16, tag=f"sq{i}", name=f"sq{i}")
              for i in range(NT)]
        sig = [sig_pool.tile([P, TLOC], bf16, tag=f"sg{i}", name=f"sg{i}")
               for i in range(NT)]

        # ---- Wk: two kt-major half-passes, 8 live PSUM groups each ----
        for hp in range(2):
            ps = [ps_pool.tile([P, 512], f32, tag=f"p{m}", name=f"psk{hp}_{m}")
                  for m in range(8)]
            for kt in range(NT):
                wt = wk_pool.tile([P, 1024], bf16, tag="wk", name=f"wk{hp}_{kt}")
                nc.scalar.dma_start(
                    wt[:], wk_d[kt * P:(kt + 1) * P, hp * 1024:(hp + 1) * 1024])
                for m in range(8):
                    nc.tensor.matmul(ps[m][:], wt[:, m * P:(m + 1) * P],
                                     hk[kt][:, HALO:],
                                     start=(kt == 0), stop=(kt == NT - 1))
            for m in range(8):
                mi = hp * 8 + m
                rr = rr_pool.tile([P, 512], bf16, tag=f"rr{mi}", name=f"rr{mi}")
                nc.scalar.activation(rr[:], ps[m][:], Act.Relu)
                nc.vector.tensor_mul(sq[mi][:], rr[:], rr[:])

        # ---- Wr: fp8 DoubleRow, group-major ----
        for g in range(4):
            pb = 4 * (g % 2)
            ps = [ps_pool.tile([P, 512], f32, tag=f"p{pb + m}", name=f"psr{g}_{m}")
                  for m in range(4)]
            for kt4 in range(NT // 4):
                wt8 = w8[g * 4 + kt4]
                for half in range(2):
                    kt2 = 2 * kt4 + half
                    for m in range(4):
                        nc.tensor.matmul(
                            ps[m][:], wt8[:, 2 * half:2 * half + 2, m * P:(m + 1) * P],
                            hr8[kt2][:, :, HALO:],
                            start=(kt2 == 0), stop=(kt2 == NT // 2 - 1),
                            perf_mode=DR)
            for m in range(4):
                nc.scalar.activation(sig[g * 4 + m][:], ps[m][:], Act.Sigmoid,
                                     scale=dec_t[:, 2 * NT:2 * NT + 1])

        # ---- Wv: group-major (m-tile groups [4,4,4,3,1] so the final
        # eviction after the last matmul is a single mul + small DMA) ----
        GRP = [(0, 4), (4, 4), (8, 4), (12, 3), (15, 1)]
        for g, (m0, gm) in enumerate(GRP):
            pb = 4 * (g % 2)
            ps = [ps_pool.tile([P, 512], f32, tag=f"p{pb + m}", name=f"psv{g}_{m}")
                  for m in range(gm)]
            for kt4 in range(NT // 4):
                wt = wv_pool.tile([P, 4, gm * P], bf16, tag=f"wv{gm}",
                                  name=f"wv{g}_{kt4}")
                nc.sync.dma_start(wt[:], wv_d[:, 4 * kt4:4 * kt4 + 4,
                                              m0 * P:(m0 + gm) * P])
                for j in range(4):
                    kt = 4 * kt4 + j
                    for m in range(gm):
                        nc.tensor.matmul(ps[m][:], wt[:, j, m * P:(m + 1) * P],
                                         sq[kt][:],
                                         start=(kt == 0), stop=(kt == NT - 1))
            m = 0
            while m < gm:
                if m + 1 < gm:
                    ot = o_pool.tile([P, 2, 512], bf16, tag="ot2", name=f"ot{g}_{m}")
                    nc.vector.tensor_mul(ot[:, 0, :], ps[m][:], sig[m0 + m][:])
                    nc.vector.tensor_mul(ot[:, 1, :], ps[m + 1][:], sig[m0 + m + 1][:])
                    nc.sync.dma_start(out_d[:, m0 + m:m0 + m + 2, :], ot[:])
                    m += 2
                else:
                    ot = o_pool.tile([P, 512], bf16, tag="ot1", name=f"ot{g}_{m}")
                    nc.vector.tensor_mul(ot[:], ps[m][:], sig[m0 + m][:])
                    nc.sync.dma_start(out_d[:, m0 + m, :], ot[:])
                    m += 1

    nc.compile()
    return nc


def _sigmoid(v):
    return 1.0 / (1.0 + np.exp(-v.astype(np.float64)))


def _prep(x, Wk, Wr, Wv, mix_k, mix_r, lxk, lxr):
    """Host-side prep: transposes, weight pre-scaling/quant, per-core slabs."""
    mk = _sigmoid(mix_k)
    mr = _sigmoid(mix_r)
    h0k = lxk.astype(np.float64) / (1.0 - mk)
    h0r = lxr.astype(np.float64) / (1.0 - mr)

    cst = np.zeros((P, 4 * NT + 2), np.float32)
    dec = cst[:, 0:2 * NT + 2]
    dec[:, 0:2 * NT:2] = mk.astype(np.float32).reshape(NT, P).T
    dec[:, 1:2 * NT:2] = mr.astype(np.float32).reshape(NT, P).T

    wk = np.ascontiguousarray(
        (Wk.T * (1.0 - mk)[:, None]).astype(ml_dtypes.bfloat16))
    # wv: [i, o] -> [p, kt, o]
    wv = np.ascontiguousarray(
        Wv.T.astype(ml_dtypes.bfloat16).reshape(NT, P, DIM).transpose(1, 0, 2))
    wrp = (Wr.T * (1.0 - mr)[:, None]).astype(np.float32)   # [i, o]
    s_w = float(240.0 / np.abs(wrp).max())
    dec[:, 2 * NT] = np.float32(1.0 / s_w)
    wr8 = np.ascontiguousarray(
        (wrp * s_w).reshape(NT // 4, 4, P, DIM).transpose(0, 2, 1, 3)
        .astype(ml_dtypes.float8_e4m3fn))

    xT = x.T.astype(np.float32)                             # [DIM, LEN]
    in_maps = []
    for c in range(NCORES):
        t0 = c * TLOC
        slab = np.empty((DIM, TS), np.float32)
        if c == 0:
            slab[:, :HALO] = 0.0
            bk = h0k * (1.0 / mk) ** HALO
            br = h0r * (1.0 / mr) ** HALO
            ini = np.empty((P, 2 * NT), np.float32)
            ini[:, 0::2] = bk.astype(np.float32).reshape(NT, P).T
            ini[:, 1::2] = br.astype(np.float32).reshape(NT, P).T
        else:
            slab[:, :HALO] = xT[:, t0 - HALO:t0]
            ini = np.zeros((P, 2 * NT), np.float32)
        cstc = cst.copy()
        cstc[:, 2 * NT + 2:] = ini
        slab[:, HALO:] = xT[:, t0:t0 + TLOC]
        img = slab.reshape(NT, P, TS).transpose(1, 0, 2).reshape(P, NT * TS)
        in_maps.append({
            "xs": np.ascontiguousarray(img.astype(ml_dtypes.bfloat16)),
            "cst": cstc, "wk": wk, "wv": wv, "wr8": wr8,
        })
    return in_maps


def kernel(x, Wk, Wr, Wv, mix_k, mix_r, last_x_mix_k, last_x_mix_r):
    x = np.asarray(x, np.float32)
    Wk = np.asarray(Wk, np.float32)
    Wr = np.asarray(Wr, np.float32)
    Wv = np.asarray(Wv, np.float32)

    if "nc" not in _cache:
        _cache["nc"] = _build()
    nc = _cache["nc"]

    in_maps = _prep(x, Wk, Wr, Wv,
                    np.asarray(mix_k, np.float32), np.asarray(mix_r, np.float32),
                    np.asarray(last_x_mix_k, np.float32),
                    np.asarray(last_x_mix_r, np.float32))
    # First execution on a cold device occasionally returns
    # NRT_EXEC_UNIT_UNRECOVERABLE; a retry has always succeeded.
    res = None
    for attempt in range(3):
        try:
            res = run_bass_kernel_spmd(nc, in_maps, core_ids=list(range(NCORES)))
            break
        except Exception:
            if attempt == 2:
                raise

    out = np.empty((LEN, DIM), np.float32)
    for c in range(NCORES):
        o = res.results[c]["out"].astype(np.float32)        # [p, m, t]
        out[c * TLOC:(c + 1) * TLOC, :] = o.transpose(1, 0, 2).reshape(DIM, TLOC).T
    return out
